# revision 9
# baseline (speedup 1.0000x reference)
"""Trainium2 Bass kernel for nn_DiscriptorMatchLoss (retrieval_knn).

loss = mean over matched pairs of (1 - cos(desc_src, desc_dst)), where a
match is dist(ps[b,n], pd[a,b,m]) <= 1 pixel AND n < m (strict upper tri).

Sharding (per hint): pair axis `a` across 8 cores; core a handles pairs
(a, b=0..7); normalized descriptors replicated (fp16). Per core:
  - dist2'[n, m] (1/64-pixel^2 units) via a K=22 fp16 PE matmul: coordinates
    split hi/mid/lo (exact fp16 chunks; products exact; row order makes
    partial sums cancel early -> near-threshold error ~2e-5) at 1 cyc/col.
    Only the strip m >= 128*i is computed for src tile i (lower tri skipped).
  - mask: diag block via DVE scalar_tensor_tensor vs a +-inf tri threshold
    directly from PSUM (with fused count); the off-diag strip is copied
    PSUM->SBUF as fp16 by ScalarE, then compared on DVE with a fast 16-bit
    tensor_scalar (fused count).
  - T[d, m] += sum_n M[n, m] * dhat_b[n, d] via fp16 PE matmuls accumulated
    in PSUM ACROSS ALL 8 PAIRS (dhat_a^T is per-core constant, so the
    masked-cos contraction distributes over pairs).
  - one final T (*) dhat_a^T reduce per core; partition-reduce via a tiny
    ones-matmul; DMA [cos_sum, count] out.
Host: loss = (sum(count) - sum(cos_sum)) / sum(count).
"""
import os
import numpy as np
import orjson
import ml_dtypes

import concourse.bass as bass
import concourse.tile as tile
from concourse import mybir
import concourse.bass_utils as bass_utils
from concourse.bass_utils import run_bass_kernel_spmd

B, N, D = 8, 1024, 256
NT = N // 128
K22 = 22
NEG = -1.0e30
THR = 1.0 / 64.0  # (radius/8)^2


# ---------------------------------------------------------------------------
# This container's walrus encodes at most 1 sync-wait per instruction (2 for
# EventSemaphore); Tile can attach more (tail drain, merged LDW+MM). Hoist
# excess waits onto standalone EventSemaphore instructions right before the
# offending instruction on the same engine (identical blocking semantics).
def _split_waits(bir: dict) -> None:
    uid = [0]

    def mk(engine, debug, waits):
        uid[0] += 1
        return {
            "debug": debug,
            "engine": engine,
            "ins": [],
            "name": f"W-fix-{uid[0]}",
            "opcode": "EventSemaphore",
            "outs": [],
            "sync_info": {"on_update": [], "on_wait": waits},
        }

    for fn in bir.get("functions", []):
        for blk in fn.get("blocks", []):
            out = []
            for ins in blk.get("instructions", []):
                si = ins.get("sync_info")
                waits = (si or {}).get("on_wait") or []
                cap = 2 if ins.get("opcode") == "EventSemaphore" else 1
                if len(waits) > cap:
                    extra = waits[cap:]
                    si["on_wait"] = waits[:cap]
                    for j in range(0, len(extra), 2):
                        out.append(mk(ins.get("engine"), ins.get("debug", 0), extra[j : j + 2]))
                out.append(ins)
            blk["instructions"] = out


class FixedBass(bass.Bass):
    def to_json_bytes(self) -> bytes:
        bir = orjson.loads(super().to_json_bytes())
        _split_waits(bir)
        return orjson.dumps(bir)


# Let walrus dedupe back-to-back LDWEIGHTS of identical stationary operands
# (bass_utils hardcodes --enable-ldw-opt=false). Results are always checked
# against the reference, and KERNEL_NO_LDW_OPT=1 restores the default.
_orig_run_command = bass_utils.run_command


def _run_command_ldwopt(argv, **kwargs):
    if os.environ.get("KERNEL_LDW_OPT"):
        argv = [
            "--enable-ldw-opt=true" if a == "--enable-ldw-opt=false" else a
            for a in argv
        ]
    return _orig_run_command(argv, **kwargs)


bass_utils.run_command = _run_command_ldwopt


def _chunks512(w):
    out = []
    off = 0
    while off < w:
        ln = min(512, w - off)
        out.append((off, ln))
        off += ln
    return out


def _build():
    f32, fp16 = mybir.dt.float32, mybir.dt.float16
    nc = FixedBass(trn_type="TRN2")
    sfeat = nc.dram_tensor("sfeat", [K22, B, N], fp16, kind="ExternalInput")
    rfeat = nc.dram_tensor("rfeat", [K22, B, N], fp16, kind="ExternalInput")
    thr = nc.dram_tensor("thr", [128, 128], f32, kind="ExternalInput")
    dh = nc.dram_tensor("dh", [128, B, NT, D], fp16, kind="ExternalInput")
    dhT = nc.dram_tensor("dhT", [128, 2, N], fp16, kind="ExternalInput")
    out = nc.dram_tensor("out", [2, 1], f32, kind="ExternalOutput")

    with tile.TileContext(nc) as tc:
        with (
            tc.tile_pool(name="const", bufs=1) as cpool,
            tc.tile_pool(name="dhp", bufs=1) as dhpool,
            tc.tile_pool(name="d16", bufs=4) as d16pool,
            tc.tile_pool(name="mask", bufs=4) as mpool,
            tc.tile_pool(name="tt", bufs=1) as ttpool,
            tc.tile_pool(name="fin", bufs=1) as fin,
            tc.tile_pool(name="pdist", bufs=2, space="PSUM") as pdp,
            tc.tile_pool(name="pT", bufs=1, space="PSUM") as pTp,
        ):
            sf = cpool.tile([K22, B, N], fp16)
            nc.sync.dma_start(sf[:], sfeat[:])
            rf = cpool.tile([K22, B, N], fp16)
            nc.sync.dma_start(rf[:], rfeat[:])
            th = cpool.tile([128, 128], f32)
            nc.sync.dma_start(th[:], thr[:])
            dT = cpool.tile([128, 2, N], fp16)
            nc.sync.dma_start(dT[:], dhT[:])
            dhb = []
            for b in range(B):
                t = dhpool.tile([128, NT, D], fp16, name=f"dh{b}")
                nc.sync.dma_start(t[:], dh[:, b, :, :])
                dhb.append(t)

            # per-strip counts: col layout [pair*16 + i] diag, [pair*16+8+i] off-diag
            count_acc = fin.tile([128, B * 2 * NT], f32)
            cos_acc = fin.tile([128, 2], f32)

            Tps = pTp.tile([128, 2, N], f32)  # accumulated over ALL pairs
            for pb in range(B):
                b = pb
                for i in range(NT):
                    m0 = 128 * i
                    w = N - m0
                    pd = pdp.tile([128, N], f32)
                    for off, ln in _chunks512(w):
                        nc.tensor.matmul(
                            pd[:, off : off + ln],
                            sf[:, b, 128 * i : 128 * (i + 1)],
                            rf[:, b, m0 + off : m0 + off + ln],
                            start=True,
                            stop=True,
                        )
                    mt = mpool.tile([128, N], fp16)
                    # diag block: tri threshold, straight from PSUM
                    nc.vector.scalar_tensor_tensor(
                        out=mt[:, 0:128],
                        in0=pd[:, 0:128],
                        scalar=1.0,
                        in1=th[:],
                        op0=mybir.AluOpType.mult,
                        op1=mybir.AluOpType.is_le,
                        accum_out=count_acc[:, pb * 16 + i : pb * 16 + i + 1],
                    )
                    if w > 128:
                        # off-diag: ScalarE casts to fp16, DVE fast compare
                        d16 = d16pool.tile([128, N], fp16)
                        nc.scalar.copy(d16[:, 0 : w - 128], pd[:, 128:w])
                        nc.vector.tensor_scalar(
                            out=mt[:, 128:w],
                            in0=d16[:, 0 : w - 128],
                            scalar1=THR,
                            scalar2=1.0,
                            op0=mybir.AluOpType.is_le,
                            op1=mybir.AluOpType.mult,
                            accum_out=count_acc[:, pb * 16 + 8 + i : pb * 16 + 8 + i + 1],
                        )
                    for c in range(2):
                        for off, ln in _chunks512(w):
                            a0 = m0 + off
                            last_i = min((a0 + ln - 1) // 128, NT - 1)
                            nc.tensor.matmul(
                                Tps[:, c, a0 : a0 + ln],
                                dhb[b][:, i, c * 128 : (c + 1) * 128],
                                mt[:, off : off + ln],
                                start=(pb == 0 and i == 0),
                                stop=(pb == B - 1 and i == last_i),
                            )

            # final: cos_sum = sum(T * dhatT_a), once per core
            for c in range(2):
                tsb = ttpool.tile([128, N], fp16, name=f"tsb{c}")
                nc.scalar.copy(tsb[:], Tps[:, c, :])
                tt = ttpool.tile([128, N], fp16, name=f"ttt{c}")
                nc.vector.scalar_tensor_tensor(
                    out=tt[:],
                    in0=tsb[:],
                    scalar=1.0,
                    in1=dT[:, c, :],
                    op0=mybir.AluOpType.mult,
                    op1=mybir.AluOpType.mult,
                    accum_out=cos_acc[:, c : c + 1],
                )

            red = fin.tile([128, 2], f32)
            nc.vector.reduce_sum(red[:, 0:1], cos_acc[:], axis=mybir.AxisListType.X)
            nc.vector.reduce_sum(red[:, 1:2], count_acc[:], axis=mybir.AxisListType.X)
            ones = fin.tile([128, 1], f32)
            nc.vector.memset(ones[:], 1.0)
            ops = pdp.tile([2, 1], f32, tag="pd")
            nc.tensor.matmul(ops[:], red[:], ones[:], start=True, stop=True)
            osb = fin.tile([2, 1], f32)
            nc.vector.tensor_copy(osb[:], ops[:])
            nc.sync.dma_start(out[:], osb[:])
    return nc


_CACHE = {}


def _get_nc():
    if "nc" not in _CACHE:
        _CACHE["nc"] = _build()
    return _CACHE["nc"]


def _split3(v):
    a = np.rint(v)
    b = (v - a).astype(np.float16)
    c = (v - a - b.astype(np.float64)).astype(np.float16)
    return a.astype(np.float16), b, c


def _splitsq(v):
    v1 = np.rint(v / 8.0) * 8.0
    v2 = (v - v1).astype(np.float16)
    v3 = (v - v1 - v2.astype(np.float64)).astype(np.float16)
    return v1.astype(np.float16), v2, v3


def _feat22(u):
    """u: [..., 2] float64 coords (1/8-pixel). Returns (F, R) each [22, ...]."""
    ax, bx, cx = _split3(u[..., 0])
    ay, by, cy = _split3(u[..., 1])
    s1, s2, s3 = _splitsq((u * u).sum(-1))
    one = np.ones_like(ax)
    m2 = np.float16(-2.0)
    Frows = [s1, ax, one, ay, s2, bx, ax, one, by, ay, s3, one,
             bx, by, ax, cx, ay, cy, bx, cx, by, cy]
    Rrows = [one, m2 * ax, s1, m2 * ay, one, m2 * ax, m2 * bx, s2,
             m2 * ay, m2 * by, one, s3, m2 * bx, m2 * by,
             m2 * cx, m2 * ax, m2 * cy, m2 * ay, m2 * cx, m2 * bx, m2 * cy, m2 * by]
    F = np.stack(Frows).astype(np.float16)
    R = np.stack(Rrows).astype(np.float16)
    return F, R


def kernel(descriptors, pts_src, pts_dst, invis_idx, height, width, **_unused):
    del invis_idx
    h = int(np.asarray(height))
    w = int(np.asarray(width))
    descriptors = np.asarray(descriptors, np.float32)
    pts_src = np.asarray(pts_src, np.float32)
    pts_dst = np.asarray(pts_dst, np.float32)

    scale = np.array([(w - 1) * 0.5, (h - 1) * 0.5], np.float32)
    ps = (pts_src + np.float32(1.0)) * scale  # fp32, matches reference
    pdst = (pts_dst + np.float32(1.0)) * scale

    us = ps.astype(np.float64) * 0.125
    ud = pdst.astype(np.float64) * 0.125
    Fs, _ = _feat22(us)  # [22, B, N]
    _, Rd = _feat22(ud)  # [22, A, B, N]
    sfeat = np.ascontiguousarray(Fs)
    rfeat_all = np.ascontiguousarray(Rd)

    d64 = descriptors.astype(np.float64)
    nrm = np.sqrt((d64 * d64).sum(-1, keepdims=True))
    dhat = (d64 / nrm).astype(np.float16)  # [B, N, D]
    dh = np.ascontiguousarray(dhat.reshape(B, NT, 128, D).transpose(2, 0, 1, 3))
    dhT_all = np.ascontiguousarray(
        dhat.transpose(0, 2, 1).reshape(B, 2, 128, N).transpose(0, 2, 1, 3)
    )

    thr = np.where(
        np.arange(128)[:, None] < np.arange(128)[None, :], np.float32(THR), np.float32(NEG)
    ).astype(np.float32)

    nc = _get_nc()
    in_maps = []
    for a in range(8):
        in_maps.append(
            {
                "sfeat": sfeat,
                "rfeat": np.ascontiguousarray(rfeat_all[:, a]),
                "thr": thr,
                "dh": dh,
                "dhT": dhT_all[a],
            }
        )
    _CACHE["last_in_maps"] = in_maps
    res = run_bass_kernel_spmd(nc, in_maps, core_ids=list(range(8)))
    cos_sum = 0.0
    count = 0.0
    for r in res.results:
        cos_sum += float(r["out"][0, 0])
        count += float(r["out"][1, 0])
    return np.float32((count - cos_sum) / count)
